# Initial kernel scaffold
#
"""Trainium2 Bass kernel for nn_CrossAttentionFusion.

Math (per batch b), all feature-major on device:
    xq = F_VNet[b]      [C=256, N=4096]   (native layout, no transpose needed)
    xk = F_Knowledge[b] [32, 4096]
    S = Kt.T @ Qt collapses: S = xk.T @ G with G = W_g @ xq + b_g,
        W_g = SCALE*(Wq.T @ Wk).T? -- precisely  G[f,q] = sum_ci wg[ci,f] xq[ci,q],
        wg = SCALE*(Wq.T @ Wk) [256, 32], b_g = SCALE*(Wk.T @ bq) [32].
        (bk is softmax-invariant -> dropped entirely.)
    U  = xk.T @ (Wv.T @ Wo.T)             [Nk, 256]  (Wo folded into V projection;
                                                      bv folds into bo_eff = bo + Wo@bv)
    E = exp(S)   (no max-subtraction: |S| small)
    Yu[co,q] = sum_k U[k,co] E[k,q];  d[q] = sum_k E[k,q]   (ones-matmul, d broadcast
                                                             across partitions by M=128)
    out = Yu * (1/d) + bo_eff + xq

Sharding: 8 cores = batch(2) x query-chunk(4 x 1024 tokens); K/V replicated
within a batch group; host slices inputs / folds weights / gathers outputs.
All matmuls in float32r: measured 227ns issue rate at N=512 (full PE rate)
with ~1e-3 relative precision. A PE warmup burst keeps the HAM clock-gate at
2.4GHz through the DMA-in phase.
"""

import os
import sys
import types

import numpy as np

for _p in (
    "/root/.axon_site",
    "/root/.axon_site/_ro/trn_rl_repo",
    "/root/.axon_site/_ro/pypackages",
    "/opt/trn_rl_repo",
):
    if os.path.isdir(_p) and _p not in sys.path:
        sys.path.append(_p)

import concourse.bass as bass  # noqa: E402,F401
import concourse.tile as tile  # noqa: E402
from concourse import bacc, mybir  # noqa: E402
from concourse.bass_utils import run_bass_kernel_spmd  # noqa: E402

F32 = mybir.dt.float32
F32R = mybir.dt.float32r
Act = mybir.ActivationFunctionType
Alu = mybir.AluOpType

B, C, CK = 2, 256, 32
N_TOK = 4096
QCH = 1024
SCALE = (256 // 4) ** (-0.5)
N_CORES = 8

CT = C // 128           # 2 c-tiles of 128
KT = N_TOK // 128       # 32 key tiles of 128
QT = QCH // 512         # 2 query tiles of 512 per core
KB = N_TOK // 512       # 8 key blocks of 512
N_WARM = int(os.environ.get("KERNEL_WARMUP", "30"))

_MM_DT = F32 if os.environ.get("KERNEL_MM_F32") else F32R


def _install_ntff_hook():
    try:
        import antenv.axon_hooks  # noqa: F401
        return True
    except ImportError:
        pass
    try:
        import antenv
        mod = types.ModuleType("antenv.axon_hooks")
        _hook = [None]
        mod.set_axon_ntff_profile_hook = lambda h: _hook.__setitem__(0, h)
        mod.get_axon_ntff_profile_hook = lambda: _hook[0]
        sys.modules["antenv.axon_hooks"] = mod
        antenv.axon_hooks = mod
        from trn_agent_boot.trn_boot import _ntff_profile_via_ctypes
        mod.set_axon_ntff_profile_hook(
            _ntff_profile_via_ctypes("/opt/axon/libaxon_pjrt.so")
        )
        return True
    except Exception:
        return False


def _build_program():
    nc = bacc.Bacc(
        "TRN2", target_bir_lowering=False, debug=False, num_devices=N_CORES
    )
    MM = _MM_DT
    xq = nc.dram_tensor("xq", [C, QCH], F32, kind="ExternalInput").ap()
    xk = nc.dram_tensor("xk", [CK, N_TOK], F32, kind="ExternalInput").ap()
    wg = nc.dram_tensor("wg", [C, CK], F32, kind="ExternalInput").ap()  # SCALE*Wq.T@Wk
    wu = nc.dram_tensor("wu", [CK, C], F32, kind="ExternalInput").ap()  # Wv.T @ Wo.T
    bg = nc.dram_tensor("bg", [CK], F32, kind="ExternalInput").ap()     # SCALE*Wk.T@bq
    boe = nc.dram_tensor("boe", [C], F32, kind="ExternalInput").ap()    # bo + Wo@bv
    out = nc.dram_tensor("out", [C, QCH], F32, kind="ExternalOutput").ap()
    out_r = out.rearrange("(t p) q -> p t q", p=128)
    xq_r = xq.rearrange("(t p) q -> p t q", p=128).bitcast(MM)
    xk_r = xk.bitcast(MM)

    with tile.TileContext(nc) as tc:
        with tc.tile_pool(name="singles", bufs=1) as singles:
            xq_sb = singles.tile([128, CT, QCH], MM)
            xk_sb = singles.tile([CK, N_TOK], MM)
            wg_sb = singles.tile([128, CT, CK], MM)
            wu_sb = singles.tile([CK, C], MM)
            bg_sb = singles.tile([CK, 1], F32)
            boe_sb = singles.tile([128, CT], F32)
            ones_f = singles.tile([128, 128], F32)
            ones_sb = singles.tile([128, 128], MM)
            g_sb = singles.tile([CK, QCH], MM)
            u_sb = singles.tile([128, KT, C], MM)

            # PE warmup burst: no data deps (memset-fed), keeps the HAM
            # clock-gate busy while input DMAs land.
            nc.vector.memset(ones_f, 1.0)
            nc.vector.tensor_copy(ones_sb, ones_f)
            with tc.tile_pool(name="warm_ps", bufs=1, space="PSUM") as wps:
                wm = wps.tile([128, 128], F32)
                for _ in range(N_WARM):
                    nc.tensor.matmul(
                        wm, lhsT=ones_sb, rhs=ones_sb, start=True, stop=True,
                        skip_group_check=True,
                    )

            # Input DMAs, smallest/most-urgent first; big tensors split so
            # multiple queues run in parallel and consumers unblock early.
            nc.sync.dma_start(out=wu_sb, in_=wu.bitcast(MM))
            nc.sync.dma_start(
                out=wg_sb, in_=wg.rearrange("(t p) f -> p t f", p=128).bitcast(MM)
            )
            nc.sync.dma_start(out=bg_sb, in_=bg[:, None])
            nc.sync.dma_start(out=boe_sb, in_=boe.rearrange("(t p) -> p t", p=128))
            for kb in range(0, KB, 2):
                ks = slice(kb * 512, (kb + 2) * 512)
                nc.sync.dma_start(out=xk_sb[:, ks], in_=xk_r[:, ks])
            for ct in range(CT):
                for qi in range(QT):
                    qsl = slice(qi * 512, (qi + 1) * 512)
                    nc.sync.dma_start(out=xq_sb[:, ct, qsl], in_=xq_r[:, ct, qsl])

            # ---- projections: G = wg.T @ xq + bg;  U = xk.T @ wu ----
            with tc.tile_pool(name="proj_ps", bufs=2, space="PSUM") as pps:
                for qi in range(QT):
                    qsl = slice(qi * 512, (qi + 1) * 512)
                    ps = pps.tile([CK, 512], F32, tag="gps")
                    for ci in range(CT):
                        nc.tensor.matmul(
                            ps,
                            lhsT=wg_sb[:, ci, :],
                            rhs=xq_sb[:, ci, qsl],
                            start=(ci == 0),
                            stop=(ci == CT - 1),
                        )
                    nc.scalar.activation(
                        out=g_sb[:, qsl], in_=ps, func=Act.Identity,
                        bias=bg_sb, scale=1.0,
                    )
                for ki in range(KT):
                    ps = pps.tile([128, C], F32, tag="ups", bufs=4)
                    nc.tensor.matmul(
                        ps,
                        lhsT=xk_sb[:, ki * 128:(ki + 1) * 128],
                        rhs=wu_sb,
                    )
                    if ki % 2 == 0:
                        nc.vector.tensor_copy(u_sb[:, ki, :], ps)
                    else:
                        nc.scalar.copy(u_sb[:, ki, :], ps)

            # ---- attention (flash over k in St=[k,q] layout) ----
            with tc.tile_pool(name="s_ps", bufs=4, space="PSUM") as sps, \
                 tc.tile_pool(name="acc_ps", bufs=1, space="PSUM") as aps, \
                 tc.tile_pool(name="epool", bufs=8) as epool, \
                 tc.tile_pool(name="epi", bufs=2) as epi:
                for qi in range(QT):
                    qsl = slice(qi * 512, (qi + 1) * 512)
                    y_ps = [
                        aps.tile([128, 512], F32, tag=f"y{h}", name=f"y_ps{h}")
                        for h in range(CT)
                    ]
                    d_ps = aps.tile([128, 512], F32, tag="d")
                    for ki in range(KT):
                        ksl = slice(ki * 128, (ki + 1) * 128)
                        sp = sps.tile([128, 512], F32, tag="s")
                        nc.tensor.matmul(
                            sp, lhsT=xk_sb[:, ksl], rhs=g_sb[:, qsl],
                        )
                        e = epool.tile([128, 512], _MM_DT, tag="e")
                        nc.scalar.activation(out=e, in_=sp, func=Act.Exp)
                        st, fin = (ki == 0), (ki == KT - 1)
                        nc.tensor.matmul(
                            d_ps, lhsT=ones_sb, rhs=e,
                            start=st, stop=fin, skip_group_check=True,
                        )
                        for h in range(CT):
                            nc.tensor.matmul(
                                y_ps[h],
                                lhsT=u_sb[:, ki, h * 128:(h + 1) * 128],
                                rhs=e, start=st, stop=fin, skip_group_check=True,
                            )
                    # epilogue: out = y * (1/d) + boe + xq, in 256-wide halves
                    # so the first multiplies overlap the second reciprocal.
                    rd = epi.tile([128, 512], F32, tag="rd")
                    scr = epi.tile([128, 256], F32, tag="scr")
                    t = [
                        epi.tile([128, 512], F32, tag=f"t{h}", name=f"t{h}")
                        for h in range(CT)
                    ]
                    for half in range(2):
                        hsl = slice(half * 256, (half + 1) * 256)
                        hslq = slice(qi * 512 + half * 256, qi * 512 + (half + 1) * 256)
                        nc.vector.reciprocal_approx_accurate(
                            out=rd[:, hsl], in_=d_ps[:, hsl], scratch=scr
                        )
                        for co in range(CT):
                            nc.vector.tensor_mul(
                                t[co][:, hsl], y_ps[co][:, hsl], rd[:, hsl]
                            )
                            nc.vector.scalar_tensor_tensor(
                                out=t[co][:, hsl], in0=t[co][:, hsl],
                                scalar=boe_sb[:, co:co + 1],
                                in1=xq_sb[:, co, hslq].bitcast(F32),
                                op0=Alu.add, op1=Alu.add,
                            )
                            nc.sync.dma_start(
                                out=out_r[:, co, hslq], in_=t[co][:, hsl]
                            )

    nc.compile()
    return nc


_NC = None


def _get_nc():
    global _NC
    if _NC is None:
        _NC = _build_program()
    return _NC


def kernel(F_VNet, F_Knowledge, Wq, bq, Wk, bk, Wv, bv, Wo, bo):
    F_VNet = np.asarray(F_VNet, dtype=np.float32)
    F_Knowledge = np.asarray(F_Knowledge, dtype=np.float32)
    Wq, bq = np.asarray(Wq, np.float32), np.asarray(bq, np.float32)
    Wv, bv = np.asarray(Wv, np.float32), np.asarray(bv, np.float32)
    Wk = np.asarray(Wk, np.float32)
    Wo, bo = np.asarray(Wo, np.float32), np.asarray(bo, np.float32)

    in_shape = F_VNet.shape
    xq_full = F_VNet.reshape(B, C, N_TOK)
    xk_full = F_Knowledge.reshape(B, CK, N_TOK)

    wg_h = np.ascontiguousarray(
        (SCALE * Wq.T.astype(np.float64) @ Wk.astype(np.float64)).astype(np.float32)
    )
    wu_h = np.ascontiguousarray(
        (Wv.T.astype(np.float64) @ Wo.T.astype(np.float64)).astype(np.float32)
    )
    bg_h = np.ascontiguousarray(SCALE * (Wk.T @ bq))
    boe_h = np.ascontiguousarray(bo + Wo @ bv)

    in_maps = []
    for core in range(N_CORES):
        b, j = divmod(core, N_CORES // B)
        in_maps.append({
            "xq": np.ascontiguousarray(xq_full[b, :, j * QCH:(j + 1) * QCH]),
            "xk": np.ascontiguousarray(xk_full[b]),
            "wg": wg_h, "wu": wu_h, "bg": bg_h, "boe": boe_h,
        })

    trace = bool(os.environ.get("KERNEL_TRACE"))
    if trace:
        _install_ntff_hook()
    nc = _get_nc()
    res = run_bass_kernel_spmd(
        nc, in_maps, core_ids=list(range(N_CORES)), trace=trace
    )
    kernel.last_results = res

    out = np.empty((B, C, N_TOK), np.float32)
    for core in range(N_CORES):
        b, j = divmod(core, N_CORES // B)
        out[b, :, j * QCH:(j + 1) * QCH] = res.results[core]["out"]
    return out.reshape(in_shape)



# revision 15
# speedup vs baseline: 1.5981x; 1.5981x over previous
"""Trainium2 Bass kernel for nn_CrossAttentionFusion.

Math (per batch b), feature-major on device:
    xq = F_VNet[b]      [C=256, N=4096]   (native layout)
    xk = F_Knowledge[b] [32, 4096]
    g  = wg.T @ xq + bg            [32, Nq]   wg = SCALE*(Wq.T@Wk) [256,32],
                                              bg = SCALE*(Wk.T@bq) [32]
    S[k,q] = sum_j xk[j,k] g[j,q]  (flash [k,q] layout; bk softmax-invariant)
    E = exp(S)  (no max-subtraction; |S| small)
    Z' = [xkT | 1]-contract:  Z'[j,q] = sum_k xkT[k,j] E[k,q]  for j<32,
         Z'[32,q] = d[q] = sum_k E[k,q]  (ones column -> denominator for free)
    out = (wu.T @ Z) * (1/d) + boe + xq,  wu = Wv.T@Wo.T, boe = bo + Wo@bv

Perf structure (vs the 112.7us 4-matmul/k-tile version):
  - 2 matmuls per k-tile (S f32r, Z bf16); the 256-wide output projection is
    applied once per q-tile to the rank-32 Z instead of every k-tile to E.
  - d rides as a 33rd stationary column of Z; broadcast to 128 partitions
    with one selector matmul, reciprocal via the fast DVE approx.
  - exp batched 3 PSUM banks per ScalarE instruction (FD=1536); ScalarE is
    the ~31us floor and gates the steady state.
  - contractions zero-padded K=32->128: every matmul runs in the (128,128)
    PE tile mode, no mode-switch drains.
  - bf16 N=256 warmup burst (MM-bound, so the HAM clock-gate actually goes
    to 8/8 -- an f32r burst is LDWEIGHTS-bound and stays cold).
  - DMAs ordered by first use; G for the second q-tile is emitted one group
    late so its xq never stalls the PE queue.

Sharding: 8 cores = batch(2) x query-chunk(4 x 1024 tokens); K/V replicated
within a batch group; host slices inputs / folds weights / gathers outputs.
"""

import os
import sys
import types

import numpy as np

for _p in (
    "/root/.axon_site",
    "/root/.axon_site/_ro/trn_rl_repo",
    "/root/.axon_site/_ro/pypackages",
    "/opt/trn_rl_repo",
):
    if os.path.isdir(_p) and _p not in sys.path:
        sys.path.append(_p)

import ml_dtypes  # noqa: E402

import concourse.bass as bass  # noqa: E402,F401
import concourse.tile as tile  # noqa: E402
from concourse import bacc, mybir  # noqa: E402
from concourse.bass_utils import run_bass_kernel_spmd  # noqa: E402

F32 = mybir.dt.float32
F32R = mybir.dt.float32r
BF16 = mybir.dt.bfloat16
Act = mybir.ActivationFunctionType
Alu = mybir.AluOpType

NP_BF16 = np.dtype(ml_dtypes.bfloat16)

B, C, CK = 2, 256, 32
N_TOK = 4096
QCH = 1024
SCALE = (256 // 4) ** (-0.5)
N_CORES = 8

CT = C // 128           # 2 c-tiles of 128
KT = N_TOK // 128       # 32 key tiles of 128
QT = QCH // 512         # 2 query tiles of 512 per core
EG = 3                  # exp group: k-tiles per ScalarE exp instruction
N_WARM = int(os.environ.get("KERNEL_WARMUP", "8"))
N_WARM_MID = int(os.environ.get("KERNEL_WARMUP_MID", "3"))


def _install_ntff_hook():
    try:
        import antenv.axon_hooks  # noqa: F401
        return True
    except ImportError:
        pass
    try:
        import antenv
        mod = types.ModuleType("antenv.axon_hooks")
        _hook = [None]
        mod.set_axon_ntff_profile_hook = lambda h: _hook.__setitem__(0, h)
        mod.get_axon_ntff_profile_hook = lambda: _hook[0]
        sys.modules["antenv.axon_hooks"] = mod
        antenv.axon_hooks = mod
        from trn_agent_boot.trn_boot import _ntff_profile_via_ctypes
        mod.set_axon_ntff_profile_hook(
            _ntff_profile_via_ctypes("/opt/axon/libaxon_pjrt.so")
        )
        return True
    except Exception:
        return False


def _build_program():
    nc = bacc.Bacc(
        "TRN2", target_bir_lowering=False, debug=False, num_devices=N_CORES
    )
    xq = nc.dram_tensor("xq", [C, QCH], F32, kind="ExternalInput").ap()
    xkf = nc.dram_tensor("xkf", [128, N_TOK], F32, kind="ExternalInput").ap()
    xkt = nc.dram_tensor("xkt", [128, KT, 128], BF16, kind="ExternalInput").ap()
    wg = nc.dram_tensor("wg", [128, CT, 128], F32, kind="ExternalInput").ap()
    wup = nc.dram_tensor("wup", [128, CT, 128], BF16, kind="ExternalInput").ap()
    bg = nc.dram_tensor("bg", [CK], F32, kind="ExternalInput").ap()
    boe = nc.dram_tensor("boe", [C], F32, kind="ExternalInput").ap()
    out = nc.dram_tensor("out", [128, CT, QCH], F32, kind="ExternalOutput").ap()
    out_r = out
    xq_r = xq.rearrange("(t p) q -> p t q", p=128).bitcast(F32R)

    groups = []  # (k0, ks) per exp group
    k0 = 0
    while k0 < KT:
        ks = min(EG, KT - k0)
        groups.append((k0, ks))
        k0 += ks
    NG = len(groups)

    with tile.TileContext(nc) as tc:
        with tc.tile_pool(name="singles", bufs=1) as singles:
            xq_sb = singles.tile([128, CT, QCH], F32R)
            xk_sb = singles.tile([128, N_TOK], F32R)
            xkt_sb = singles.tile([128, KT, 128], BF16)
            wg_sb = singles.tile([128, CT, 128], F32R)
            wup_sb = singles.tile([128, CT, 128], BF16)
            bg_sb = singles.tile([CK, 1], F32)
            boe_sb = singles.tile([128, CT], F32)
            g_sb = singles.tile([128, QCH], F32R)
            zf = singles.tile([128, QCH], F32)
            wj = singles.tile([128, 256], BF16)
            sel_f = singles.tile([128, 128], F32)
            sel_b = singles.tile([128, 128], BF16)
            warm_in = singles.tile([1, 8], F32)
            warm_e = singles.tile([1, 8], F32)
            z_sb = singles.tile([128, QT, 512], BF16)
            zn_sb = singles.tile([128, 512], BF16)
            r33 = singles.tile([CK + 1, 512], F32)

            # Constants; zf zero-fills the padded rows of g_sb (memset can't
            # write f32r directly -- ISA restriction -- a DVE copy can).
            nc.vector.memset(wj, 0.0)
            nc.vector.memset(zf, 0.0)
            nc.vector.memset(sel_f, 0.0)
            nc.vector.memset(sel_f[CK:CK + 1, :], 1.0)
            nc.vector.memset(warm_in, 0.0)
            nc.vector.tensor_copy(g_sb, zf)
            nc.vector.tensor_copy(zn_sb, zf.bitcast(BF16)[:, 0:512])
            nc.vector.tensor_copy(sel_b, sel_f)

            # Input DMAs: each dma_start lands on ONE queue (~45GB/s),
            # so first-needed tensors are split into <=128KB chunks spread
            # across queues; bulk follows in larger pieces.
            nc.sync.dma_start(out=bg_sb, in_=bg[:, None])
            for ci in range(CT):
                nc.sync.dma_start(
                    out=wg_sb[:, ci, :], in_=wg[:, ci, :].bitcast(F32R)
                )
            for qh in range(4):
                qs = slice(qh * 256, (qh + 1) * 256)
                for ci in range(CT):
                    nc.sync.dma_start(
                        out=xq_sb[:, ci, qs], in_=xq_r[:, ci, qs]
                    )
            for kb in range(4):
                ks_ = slice(kb * 256, (kb + 1) * 256)
                nc.sync.dma_start(
                    out=xk_sb[:, ks_], in_=xkf[:, ks_].bitcast(F32R)
                )
            nc.sync.dma_start(out=xkt_sb[:, 0:4, :], in_=xkt[:, 0:4, :])
            nc.sync.dma_start(out=xkt_sb[:, 4:8, :], in_=xkt[:, 4:8, :])
            nc.sync.dma_start(out=wup_sb, in_=wup)
            nc.sync.dma_start(out=boe_sb, in_=boe.rearrange("(t p) -> p t", p=128))
            for kb in range(2, 8):
                ks_ = slice(kb * 512, (kb + 1) * 512)
                nc.sync.dma_start(
                    out=xk_sb[:, ks_], in_=xkf[:, ks_].bitcast(F32R)
                )
                kt_ = slice(kb * 4, (kb + 1) * 4)
                nc.sync.dma_start(out=xkt_sb[:, kt_, :], in_=xkt[:, kt_, :])

            # ACT exp-table load (~2.7us) overlaps the DMA-in phase.
            nc.scalar.activation(out=warm_e, in_=warm_in, func=Act.Exp)

            with tc.tile_pool(name="sps", bufs=2, space="PSUM") as sps, \
                 tc.tile_pool(name="zps", bufs=1, space="PSUM") as zps, \
                 tc.tile_pool(name="eps", bufs=1, space="PSUM") as eps, \
                 tc.tile_pool(name="epool", bufs=3) as epool, \
                 tc.tile_pool(name="tpool", bufs=4) as tpool:

                # MM-bound bf16 warmup: the HAM clock-gate needs ~3.4us of
                # gapless PE activity before it opens to 2.4GHz, so the
                # burst uses the (idle) double-buffered sps banks -- a
                # single-bank ring serializes on the WAW drain and never
                # warms -- and seamlessly abuts G and the first S groups.
                def emit_warm(n):
                    for _ in range(n):
                        wm = sps.tile([128, EG, 512], F32, tag="s", name="wm")
                        nc.tensor.matmul(
                            wm[:, 0, 0:256], lhsT=wj[:, 0:128], rhs=wj,
                            start=True, stop=True, skip_group_check=True,
                        )

                def emit_g(qi):
                    # G = wg.T @ xq (+bg) -> g rows 0..31 (rows 32+ stay 0)
                    qsl = slice(qi * 512, (qi + 1) * 512)
                    pool, tg = (zps, "z") if qi == 0 else (eps, "e")
                    gp = pool.tile([128, 512], F32, tag=tg, name=f"gp{qi}")
                    for ci in range(CT):
                        nc.tensor.matmul(
                            gp, lhsT=wg_sb[:, ci, :],
                            rhs=xq_sb[:, ci, qsl],
                            start=(ci == 0), stop=(ci == CT - 1),
                        )
                    nc.vector.tensor_scalar(
                        out=g_sb[0:CK, qsl], in0=gp[0:CK, :],
                        scalar1=bg_sb, scalar2=None, op0=Alu.add,
                    )

                def emit_sgroup(qi, gi):
                    k0, ks = groups[gi]
                    qsl = slice(qi * 512, (qi + 1) * 512)
                    sp = sps.tile([128, EG, 512], F32, tag="s", name="sp")
                    for i in range(ks):
                        nc.tensor.matmul(
                            sp[:, i, :],
                            lhsT=xk_sb[:, (k0 + i) * 128:(k0 + i + 1) * 128],
                            rhs=g_sb[:, qsl],
                            start=True, stop=True, skip_group_check=True,
                        )
                    return sp

                def emit_expz(qi, gi, sp, z_ps):
                    k0, ks = groups[gi]
                    e = epool.tile([128, EG, 512], BF16, tag="e", name="e")
                    nc.scalar.activation(
                        out=e[:, 0:ks, :], in_=sp[:, 0:ks, :], func=Act.Exp
                    )
                    for i in range(ks):
                        nc.tensor.matmul(
                            z_ps,
                            lhsT=xkt_sb[:, k0 + i, :],
                            rhs=e[:, i, :],
                            start=(k0 + i == 0), stop=(k0 + i == KT - 1),
                            skip_group_check=True,
                        )

                def emit_epilogue(qi, z_ps):
                    qsl = slice(qi * 512, (qi + 1) * 512)
                    last = qi == QT - 1
                    # z rows 33..127 are exact zeros (zero-padded stationary)
                    nc.vector.tensor_copy(z_sb[:, qi, :], z_ps)
                    d128 = eps.tile([128, 512], F32, tag="e", name="d128")
                    nc.tensor.matmul(
                        d128, lhsT=sel_b, rhs=z_sb[:, qi, :],
                        start=True, stop=True, skip_group_check=True,
                    )
                    nc.vector.reciprocal_approx_fast(
                        out=r33, in_=d128[0:CK + 1, :]
                    )
                    # normalize the rank-33 z once instead of the two
                    # 128-wide y tiles (zn rows 33..127 stay zero)
                    nc.vector.tensor_mul(
                        zn_sb[0:CK + 1, :], z_sb[0:CK + 1, qi, :], r33,
                    )
                    for co in range(CT):
                        # y1 rides the zps bank (free after the z copy) so it
                        # never waits on stt0 draining the eps bank
                        pool, tg = (eps, "e") if co == 0 else (zps, "z")
                        yp = pool.tile([128, 512], F32, tag=tg, name=f"y{co}")
                        nc.tensor.matmul(
                            yp, lhsT=wup_sb[:, co, :], rhs=zn_sb,
                            start=True, stop=True, skip_group_check=True,
                        )
                        t = tpool.tile([128, 512], F32, tag=f"t{co}",
                                       name=f"t{co}")
                        nc.vector.scalar_tensor_tensor(
                            out=t, in0=yp, scalar=boe_sb[:, co:co + 1],
                            in1=xq_sb[:, co, qsl].bitcast(F32),
                            op0=Alu.add, op1=Alu.add,
                        )
                        if last:
                            # 4-way queue-parallel drain of the final tiles
                            for h in range(4):
                                hs = slice(h * 128, (h + 1) * 128)
                                hq = slice(qi * 512 + h * 128,
                                           qi * 512 + (h + 1) * 128)
                                nc.sync.dma_start(
                                    out=out_r[:, co, hq], in_=t[:, hs]
                                )
                        else:
                            nc.sync.dma_start(out=out_r[:, co, qsl], in_=t)

                emit_warm(N_WARM)
                emit_g(0)
                emit_warm(N_WARM_MID)
                emit_g(1)
                # flat (qi, gi) schedule with one-group lookahead across the
                # q-tile boundary so ACT never drains at the transition
                seq = [(qi, gi) for qi in range(QT) for gi in range(NG)]
                zt = {}
                zt[0] = zps.tile([128, 512], F32, tag="z", name="z0")
                sp = emit_sgroup(*seq[0])
                for idx, (qi, gi) in enumerate(seq):
                    if idx + 1 < len(seq):
                        nqi, ngi = seq[idx + 1]
                        if ngi == 0:
                            zt[nqi] = zps.tile(
                                [128, 512], F32, tag="z", name=f"z{nqi}"
                            )
                        sp_next = emit_sgroup(nqi, ngi)
                    emit_expz(qi, gi, sp, zt[qi])
                    if idx + 1 < len(seq):
                        sp = sp_next
                    # previous q-tile's epilogue right after this q-tile's
                    # first exp is queued
                    if gi == 0 and qi > 0:
                        emit_epilogue(qi - 1, zt[qi - 1])
                emit_epilogue(QT - 1, zt[QT - 1])

    nc.compile()
    return nc


_NC = None


def _get_nc():
    global _NC
    if _NC is None:
        _NC = _build_program()
    return _NC


def kernel(F_VNet, F_Knowledge, Wq, bq, Wk, bk, Wv, bv, Wo, bo):
    F_VNet = np.asarray(F_VNet, dtype=np.float32)
    F_Knowledge = np.asarray(F_Knowledge, dtype=np.float32)
    Wq, bq = np.asarray(Wq, np.float32), np.asarray(bq, np.float32)
    Wv, bv = np.asarray(Wv, np.float32), np.asarray(bv, np.float32)
    Wk = np.asarray(Wk, np.float32)
    Wo, bo = np.asarray(Wo, np.float32), np.asarray(bo, np.float32)

    in_shape = F_VNet.shape
    xq_full = F_VNet.reshape(B, C, N_TOK)
    xk_full = F_Knowledge.reshape(B, CK, N_TOK)

    wg_h = (SCALE * Wq.T.astype(np.float64) @ Wk.astype(np.float64)).astype(
        np.float32
    )  # [256, 32]
    wu_h = (Wv.T.astype(np.float64) @ Wo.T.astype(np.float64)).astype(
        np.float32
    )  # [32, 256]
    bg_h = np.ascontiguousarray(SCALE * (Wk.T @ bq))
    boe_h = np.ascontiguousarray((bo + Wo @ bv).reshape(CT, 128).T)  # [128, CT]

    wg_pad = np.zeros((128, CT, 128), np.float32)
    wg_pad[:, :, 0:CK] = wg_h.reshape(CT, 128, CK).transpose(1, 0, 2)
    wup_pad = np.zeros((128, CT, 128), NP_BF16)
    for co in range(CT):
        wup_pad[0:CK, co, :] = wu_h[:, co * 128:(co + 1) * 128]

    per_b = {}
    for b in range(B):
        xk_pad = np.zeros((128, N_TOK), np.float32)
        xk_pad[0:CK, :] = xk_full[b]
        xkt_pad = np.zeros((128, KT, 128), NP_BF16)
        # [p, ki, j] = xk[j, ki*128+p] for j<32; 1.0 at j==32
        xkt_pad[:, :, 0:CK] = (
            xk_full[b].T.reshape(KT, 128, CK).transpose(1, 0, 2)
        )
        xkt_pad[:, :, CK] = 1.0
        per_b[b] = (xk_pad, np.ascontiguousarray(xkt_pad))

    in_maps = []
    for core in range(N_CORES):
        b, j = divmod(core, N_CORES // B)
        xk_pad, xkt_pad = per_b[b]
        in_maps.append({
            "xq": np.ascontiguousarray(xq_full[b, :, j * QCH:(j + 1) * QCH]),
            "xkf": xk_pad, "xkt": xkt_pad,
            "wg": wg_pad, "wup": wup_pad,
            "bg": bg_h, "boe": boe_h,
        })

    trace = bool(os.environ.get("KERNEL_TRACE"))
    if trace:
        _install_ntff_hook()
    nc = _get_nc()
    res = run_bass_kernel_spmd(
        nc, in_maps, core_ids=list(range(N_CORES)), trace=trace
    )
    kernel.last_results = res

    out = np.empty((B, C, N_TOK), np.float32)
    for core in range(N_CORES):
        b, j = divmod(core, N_CORES // B)
        # device layout [128, CT, QCH] -> [C, QCH]
        o = res.results[core]["out"].transpose(1, 0, 2).reshape(C, QCH)
        out[b, :, j * QCH:(j + 1) * QCH] = o
    return out.reshape(in_shape)


# revision 16
# speedup vs baseline: 1.6561x; 1.0363x over previous
"""Trainium2 Bass kernel for nn_CrossAttentionFusion.

Math (per batch b), feature-major on device:
    xq = F_VNet[b]      [C=256, N=4096]   (native layout)
    xk = F_Knowledge[b] [32, 4096]
    g  = wg.T @ xq + bg            [32, Nq]   wg = SCALE*(Wq.T@Wk) [256,32],
                                              bg = SCALE*(Wk.T@bq) [32]
    S[k,q] = sum_j xk[j,k] g[j,q]  (flash [k,q] layout; bk softmax-invariant)
    E = exp(S)  (no max-subtraction; |S| small)
    Z' = [xkT | 1]-contract:  Z'[j,q] = sum_k xkT[k,j] E[k,q]  for j<32,
         Z'[32,q] = d[q] = sum_k E[k,q]  (ones column -> denominator for free)
    out = (wu.T @ Z) * (1/d) + boe + xq,  wu = Wv.T@Wo.T, boe = bo + Wo@bv

Perf structure (vs the 112.7us 4-matmul/k-tile version):
  - 2 matmuls per k-tile (S f32r, Z bf16); the 256-wide output projection is
    applied once per q-tile to the rank-32 Z instead of every k-tile to E.
  - d rides as a 33rd stationary column of Z; broadcast to 128 partitions
    with one selector matmul, reciprocal via the fast DVE approx.
  - exp batched 3 PSUM banks per ScalarE instruction (FD=1536); ScalarE is
    the ~31us floor and gates the steady state.
  - contractions zero-padded K=32->128: every matmul runs in the (128,128)
    PE tile mode, no mode-switch drains.
  - bf16 N=256 warmup burst (MM-bound, so the HAM clock-gate actually goes
    to 8/8 -- an f32r burst is LDWEIGHTS-bound and stays cold).
  - DMAs ordered by first use; G for the second q-tile is emitted one group
    late so its xq never stalls the PE queue.

Sharding: 8 cores = batch(2) x query-chunk(4 x 1024 tokens); K/V replicated
within a batch group; host slices inputs / folds weights / gathers outputs.
"""

import os
import sys
import types

import numpy as np

for _p in (
    "/root/.axon_site",
    "/root/.axon_site/_ro/trn_rl_repo",
    "/root/.axon_site/_ro/pypackages",
    "/opt/trn_rl_repo",
):
    if os.path.isdir(_p) and _p not in sys.path:
        sys.path.append(_p)

import ml_dtypes  # noqa: E402

import concourse.bass as bass  # noqa: E402,F401
import concourse.tile as tile  # noqa: E402
from concourse import bacc, mybir  # noqa: E402
from concourse.bass_utils import run_bass_kernel_spmd  # noqa: E402

F32 = mybir.dt.float32
F32R = mybir.dt.float32r
BF16 = mybir.dt.bfloat16
Act = mybir.ActivationFunctionType
Alu = mybir.AluOpType

NP_BF16 = np.dtype(ml_dtypes.bfloat16)

B, C, CK = 2, 256, 32
N_TOK = 4096
QCH = 1024
SCALE = (256 // 4) ** (-0.5)
N_CORES = 8

CT = C // 128           # 2 c-tiles of 128
KT = N_TOK // 128       # 32 key tiles of 128
QT = QCH // 512         # 2 query tiles of 512 per core
EG = 3                  # exp group: k-tiles per ScalarE exp instruction
N_WARM = int(os.environ.get("KERNEL_WARMUP", "8"))
N_WARM_MID = int(os.environ.get("KERNEL_WARMUP_MID", "3"))


def _install_ntff_hook():
    try:
        import antenv.axon_hooks  # noqa: F401
        return True
    except ImportError:
        pass
    try:
        import antenv
        mod = types.ModuleType("antenv.axon_hooks")
        _hook = [None]
        mod.set_axon_ntff_profile_hook = lambda h: _hook.__setitem__(0, h)
        mod.get_axon_ntff_profile_hook = lambda: _hook[0]
        sys.modules["antenv.axon_hooks"] = mod
        antenv.axon_hooks = mod
        from trn_agent_boot.trn_boot import _ntff_profile_via_ctypes
        mod.set_axon_ntff_profile_hook(
            _ntff_profile_via_ctypes("/opt/axon/libaxon_pjrt.so")
        )
        return True
    except Exception:
        return False


def _build_program():
    nc = bacc.Bacc(
        "TRN2", target_bir_lowering=False, debug=False, num_devices=N_CORES
    )
    xq = nc.dram_tensor("xq", [C, QCH], F32, kind="ExternalInput").ap()
    xkf = nc.dram_tensor("xkf", [128, N_TOK], F32, kind="ExternalInput").ap()
    xkt = nc.dram_tensor("xkt", [128, KT, 128], BF16, kind="ExternalInput").ap()
    wg = nc.dram_tensor("wg", [128, CT, 128], F32, kind="ExternalInput").ap()
    wup = nc.dram_tensor("wup", [128, CT, 128], BF16, kind="ExternalInput").ap()
    bg = nc.dram_tensor("bg", [CK], F32, kind="ExternalInput").ap()
    boe = nc.dram_tensor("boe", [C], F32, kind="ExternalInput").ap()
    out = nc.dram_tensor("out", [128, CT, QCH], F32, kind="ExternalOutput").ap()
    out_r = out
    xq_r = xq.rearrange("(t p) q -> p t q", p=128).bitcast(F32R)

    groups = []  # (k0, ks) per exp group
    k0 = 0
    while k0 < KT:
        ks = min(EG, KT - k0)
        groups.append((k0, ks))
        k0 += ks
    NG = len(groups)

    with tile.TileContext(nc) as tc:
        with tc.tile_pool(name="singles", bufs=1) as singles:
            xq_sb = singles.tile([128, CT, QCH], F32R)
            xk_sb = singles.tile([128, N_TOK], F32R)
            xkt_sb = singles.tile([128, KT, 128], BF16)
            wg_sb = singles.tile([128, CT, 128], F32R)
            wup_sb = singles.tile([128, CT, 128], BF16)
            bg_sb = singles.tile([CK, 1], F32)
            boe_sb = singles.tile([128, CT], F32)
            g_sb = singles.tile([128, QCH], F32R)
            zf = singles.tile([128, QCH], F32)
            wj = singles.tile([128, 256], BF16)
            sel_f = singles.tile([128, 128], F32)
            sel_b = singles.tile([128, 128], BF16)
            warm_in = singles.tile([1, 8], F32)
            warm_e = singles.tile([1, 8], F32)
            z_sb = singles.tile([128, QT, 512], BF16)
            zn_sb = singles.tile([128, 512], BF16)
            r33 = singles.tile([CK + 1, 512], F32)

            # Constants; zf zero-fills the padded rows of g_sb (memset can't
            # write f32r directly -- ISA restriction -- a DVE copy can).
            nc.vector.memset(wj, 0.0)
            nc.vector.memset(zf, 0.0)
            nc.vector.memset(sel_f, 0.0)
            nc.vector.memset(sel_f[CK:CK + 1, :], 1.0)
            nc.vector.memset(warm_in, 0.0)
            nc.vector.tensor_copy(g_sb, zf)
            nc.vector.tensor_copy(zn_sb, zf.bitcast(BF16)[:, 0:512])
            nc.vector.tensor_copy(sel_b, sel_f)

            # Input DMAs ordered by first use. Fewer, larger dma_starts
            # win: per-dma_start issue overhead dominates the front, the
            # transfer itself runs at aggregate bandwidth.
            nc.sync.dma_start(out=bg_sb, in_=bg[:, None])
            nc.sync.dma_start(out=wg_sb, in_=wg.bitcast(F32R))
            for ci in range(CT):
                nc.sync.dma_start(
                    out=xq_sb[:, ci, 0:512], in_=xq_r[:, ci, 0:512]
                )
            nc.sync.dma_start(
                out=xk_sb[:, 0:1024], in_=xkf[:, 0:1024].bitcast(F32R)
            )
            nc.sync.dma_start(out=xkt_sb[:, 0:8, :], in_=xkt[:, 0:8, :])
            for ci in range(CT):
                nc.sync.dma_start(
                    out=xq_sb[:, ci, 512:1024], in_=xq_r[:, ci, 512:1024]
                )
            for kb in range(1, 4):
                ks_ = slice(kb * 1024, (kb + 1) * 1024)
                nc.sync.dma_start(
                    out=xk_sb[:, ks_], in_=xkf[:, ks_].bitcast(F32R)
                )
                kt_ = slice(kb * 8, (kb + 1) * 8)
                nc.sync.dma_start(out=xkt_sb[:, kt_, :], in_=xkt[:, kt_, :])
            nc.sync.dma_start(out=wup_sb, in_=wup)
            nc.sync.dma_start(out=boe_sb, in_=boe.rearrange("(t p) -> p t", p=128))

            # ACT exp-table load (~2.7us) overlaps the DMA-in phase.
            nc.scalar.activation(out=warm_e, in_=warm_in, func=Act.Exp)

            with tc.tile_pool(name="sps", bufs=2, space="PSUM") as sps, \
                 tc.tile_pool(name="zps", bufs=1, space="PSUM") as zps, \
                 tc.tile_pool(name="eps", bufs=1, space="PSUM") as eps, \
                 tc.tile_pool(name="epool", bufs=3) as epool, \
                 tc.tile_pool(name="tpool", bufs=4) as tpool:

                # MM-bound bf16 warmup: the HAM clock-gate needs ~3.4us of
                # gapless PE activity before it opens to 2.4GHz, so the
                # burst uses the (idle) double-buffered sps banks -- a
                # single-bank ring serializes on the WAW drain and never
                # warms -- and seamlessly abuts G and the first S groups.
                def emit_warm(n):
                    for _ in range(n):
                        wm = sps.tile([128, EG, 512], F32, tag="s", name="wm")
                        nc.tensor.matmul(
                            wm[:, 0, 0:256], lhsT=wj[:, 0:128], rhs=wj,
                            start=True, stop=True, skip_group_check=True,
                        )

                def emit_g(qi):
                    # G = wg.T @ xq (+bg) -> g rows 0..31 (rows 32+ stay 0)
                    qsl = slice(qi * 512, (qi + 1) * 512)
                    pool, tg = (zps, "z") if qi == 0 else (eps, "e")
                    gp = pool.tile([128, 512], F32, tag=tg, name=f"gp{qi}")
                    for ci in range(CT):
                        nc.tensor.matmul(
                            gp, lhsT=wg_sb[:, ci, :],
                            rhs=xq_sb[:, ci, qsl],
                            start=(ci == 0), stop=(ci == CT - 1),
                        )
                    nc.vector.tensor_scalar(
                        out=g_sb[0:CK, qsl], in0=gp[0:CK, :],
                        scalar1=bg_sb, scalar2=None, op0=Alu.add,
                    )

                def emit_sgroup(qi, gi):
                    k0, ks = groups[gi]
                    qsl = slice(qi * 512, (qi + 1) * 512)
                    sp = sps.tile([128, EG, 512], F32, tag="s", name="sp")
                    for i in range(ks):
                        nc.tensor.matmul(
                            sp[:, i, :],
                            lhsT=xk_sb[:, (k0 + i) * 128:(k0 + i + 1) * 128],
                            rhs=g_sb[:, qsl],
                            start=True, stop=True, skip_group_check=True,
                        )
                    return sp

                def emit_expz(qi, gi, sp, z_ps):
                    k0, ks = groups[gi]
                    e = epool.tile([128, EG, 512], BF16, tag="e", name="e")
                    nc.scalar.activation(
                        out=e[:, 0:ks, :], in_=sp[:, 0:ks, :], func=Act.Exp
                    )
                    for i in range(ks):
                        nc.tensor.matmul(
                            z_ps,
                            lhsT=xkt_sb[:, k0 + i, :],
                            rhs=e[:, i, :],
                            start=(k0 + i == 0), stop=(k0 + i == KT - 1),
                            skip_group_check=True,
                        )

                def emit_epilogue(qi, z_ps):
                    qsl = slice(qi * 512, (qi + 1) * 512)
                    last = qi == QT - 1
                    # z rows 33..127 are exact zeros (zero-padded stationary)
                    nc.vector.tensor_copy(z_sb[:, qi, :], z_ps)
                    d128 = eps.tile([128, 512], F32, tag="e", name="d128")
                    nc.tensor.matmul(
                        d128, lhsT=sel_b, rhs=z_sb[:, qi, :],
                        start=True, stop=True, skip_group_check=True,
                    )
                    nc.vector.reciprocal_approx_fast(
                        out=r33, in_=d128[0:CK + 1, :]
                    )
                    # normalize the rank-33 z once instead of the two
                    # 128-wide y tiles (zn rows 33..127 stay zero)
                    nc.vector.tensor_mul(
                        zn_sb[0:CK + 1, :], z_sb[0:CK + 1, qi, :], r33,
                    )
                    for co in range(CT):
                        # y1 rides the zps bank (free after the z copy) so it
                        # never waits on stt0 draining the eps bank
                        pool, tg = (eps, "e") if co == 0 else (zps, "z")
                        yp = pool.tile([128, 512], F32, tag=tg, name=f"y{co}")
                        nc.tensor.matmul(
                            yp, lhsT=wup_sb[:, co, :], rhs=zn_sb,
                            start=True, stop=True, skip_group_check=True,
                        )
                        t = tpool.tile([128, 512], F32, tag=f"t{co}",
                                       name=f"t{co}")
                        nc.vector.scalar_tensor_tensor(
                            out=t, in0=yp, scalar=boe_sb[:, co:co + 1],
                            in1=xq_sb[:, co, qsl].bitcast(F32),
                            op0=Alu.add, op1=Alu.add,
                        )
                        if last:
                            # 4-way queue-parallel drain of the final tiles
                            for h in range(4):
                                hs = slice(h * 128, (h + 1) * 128)
                                hq = slice(qi * 512 + h * 128,
                                           qi * 512 + (h + 1) * 128)
                                nc.sync.dma_start(
                                    out=out_r[:, co, hq], in_=t[:, hs]
                                )
                        else:
                            nc.sync.dma_start(out=out_r[:, co, qsl], in_=t)

                emit_warm(N_WARM)
                emit_g(0)
                emit_warm(N_WARM_MID)
                emit_g(1)
                # flat (qi, gi) schedule with one-group lookahead across the
                # q-tile boundary so ACT never drains at the transition
                seq = [(qi, gi) for qi in range(QT) for gi in range(NG)]
                zt = {}
                zt[0] = zps.tile([128, 512], F32, tag="z", name="z0")
                sp = emit_sgroup(*seq[0])
                for idx, (qi, gi) in enumerate(seq):
                    if idx + 1 < len(seq):
                        nqi, ngi = seq[idx + 1]
                        if ngi == 0:
                            zt[nqi] = zps.tile(
                                [128, 512], F32, tag="z", name=f"z{nqi}"
                            )
                        sp_next = emit_sgroup(nqi, ngi)
                    emit_expz(qi, gi, sp, zt[qi])
                    if idx + 1 < len(seq):
                        sp = sp_next
                    # previous q-tile's epilogue right after this q-tile's
                    # first exp is queued
                    if gi == 0 and qi > 0:
                        emit_epilogue(qi - 1, zt[qi - 1])
                emit_epilogue(QT - 1, zt[QT - 1])

    nc.compile()
    return nc


_NC = None


def _get_nc():
    global _NC
    if _NC is None:
        _NC = _build_program()
    return _NC


def kernel(F_VNet, F_Knowledge, Wq, bq, Wk, bk, Wv, bv, Wo, bo):
    F_VNet = np.asarray(F_VNet, dtype=np.float32)
    F_Knowledge = np.asarray(F_Knowledge, dtype=np.float32)
    Wq, bq = np.asarray(Wq, np.float32), np.asarray(bq, np.float32)
    Wv, bv = np.asarray(Wv, np.float32), np.asarray(bv, np.float32)
    Wk = np.asarray(Wk, np.float32)
    Wo, bo = np.asarray(Wo, np.float32), np.asarray(bo, np.float32)

    in_shape = F_VNet.shape
    xq_full = F_VNet.reshape(B, C, N_TOK)
    xk_full = F_Knowledge.reshape(B, CK, N_TOK)

    wg_h = (SCALE * Wq.T.astype(np.float64) @ Wk.astype(np.float64)).astype(
        np.float32
    )  # [256, 32]
    wu_h = (Wv.T.astype(np.float64) @ Wo.T.astype(np.float64)).astype(
        np.float32
    )  # [32, 256]
    bg_h = np.ascontiguousarray(SCALE * (Wk.T @ bq))
    boe_h = np.ascontiguousarray((bo + Wo @ bv).reshape(CT, 128).T)  # [128, CT]

    wg_pad = np.zeros((128, CT, 128), np.float32)
    wg_pad[:, :, 0:CK] = wg_h.reshape(CT, 128, CK).transpose(1, 0, 2)
    wup_pad = np.zeros((128, CT, 128), NP_BF16)
    for co in range(CT):
        wup_pad[0:CK, co, :] = wu_h[:, co * 128:(co + 1) * 128]

    per_b = {}
    for b in range(B):
        xk_pad = np.zeros((128, N_TOK), np.float32)
        xk_pad[0:CK, :] = xk_full[b]
        xkt_pad = np.zeros((128, KT, 128), NP_BF16)
        # [p, ki, j] = xk[j, ki*128+p] for j<32; 1.0 at j==32
        xkt_pad[:, :, 0:CK] = (
            xk_full[b].T.reshape(KT, 128, CK).transpose(1, 0, 2)
        )
        xkt_pad[:, :, CK] = 1.0
        per_b[b] = (xk_pad, np.ascontiguousarray(xkt_pad))

    in_maps = []
    for core in range(N_CORES):
        b, j = divmod(core, N_CORES // B)
        xk_pad, xkt_pad = per_b[b]
        in_maps.append({
            "xq": np.ascontiguousarray(xq_full[b, :, j * QCH:(j + 1) * QCH]),
            "xkf": xk_pad, "xkt": xkt_pad,
            "wg": wg_pad, "wup": wup_pad,
            "bg": bg_h, "boe": boe_h,
        })

    trace = bool(os.environ.get("KERNEL_TRACE"))
    if trace:
        _install_ntff_hook()
    nc = _get_nc()
    res = run_bass_kernel_spmd(
        nc, in_maps, core_ids=list(range(N_CORES)), trace=trace
    )
    kernel.last_results = res

    out = np.empty((B, C, N_TOK), np.float32)
    for core in range(N_CORES):
        b, j = divmod(core, N_CORES // B)
        # device layout [128, CT, QCH] -> [C, QCH]
        o = res.results[core]["out"].transpose(1, 0, 2).reshape(C, QCH)
        out[b, :, j * QCH:(j + 1) * QCH] = o
    return out.reshape(in_shape)


# revision 19
# speedup vs baseline: 1.6922x; 1.0218x over previous
"""Trainium2 Bass kernel for nn_CrossAttentionFusion.

Math (per batch b), feature-major on device:
    xq = F_VNet[b]      [C=256, N=4096]   (native layout)
    xk = F_Knowledge[b] [32, 4096]
    g  = wg.T @ xq + bg            [32, Nq]   wg = SCALE*(Wq.T@Wk) [256,32],
                                              bg = SCALE*(Wk.T@bq) [32]
    S[k,q] = sum_j xk[j,k] g[j,q]  (flash [k,q] layout; bk softmax-invariant)
    E = exp(S)  (no max-subtraction; |S| small)
    Z' = [xkT | 1]-contract:  Z'[j,q] = sum_k xkT[k,j] E[k,q]  for j<32,
         Z'[32,q] = d[q] = sum_k E[k,q]  (ones column -> denominator for free)
    out = (wu.T @ Z) * (1/d) + boe + xq,  wu = Wv.T@Wo.T, boe = bo + Wo@bv

Perf structure (vs the 112.7us 4-matmul/k-tile version):
  - 2 matmuls per k-tile (S f32r, Z bf16); the 256-wide output projection is
    applied once per q-tile to the rank-32 Z instead of every k-tile to E.
  - d rides as a 33rd stationary column of Z; broadcast to 128 partitions
    with one selector matmul, reciprocal via the fast DVE approx.
  - exp batched 3 PSUM banks per ScalarE instruction (FD=1536); ScalarE is
    the ~31us floor and gates the steady state.
  - contractions zero-padded K=32->128: every matmul runs in the (128,128)
    PE tile mode, no mode-switch drains.
  - bf16 N=256 warmup burst (MM-bound, so the HAM clock-gate actually goes
    to 8/8 -- an f32r burst is LDWEIGHTS-bound and stays cold).
  - DMAs ordered by first use; G for the second q-tile is emitted one group
    late so its xq never stalls the PE queue.

Sharding: 8 cores = batch(2) x query-chunk(4 x 1024 tokens); K/V replicated
within a batch group; host slices inputs / folds weights / gathers outputs.
"""

import os
import sys
import types

import numpy as np

for _p in (
    "/root/.axon_site",
    "/root/.axon_site/_ro/trn_rl_repo",
    "/root/.axon_site/_ro/pypackages",
    "/opt/trn_rl_repo",
):
    if os.path.isdir(_p) and _p not in sys.path:
        sys.path.append(_p)

import ml_dtypes  # noqa: E402

import concourse.bass as bass  # noqa: E402,F401
import concourse.tile as tile  # noqa: E402
from concourse import bacc, mybir  # noqa: E402
from concourse.bass_utils import run_bass_kernel_spmd  # noqa: E402

F32 = mybir.dt.float32
F32R = mybir.dt.float32r
BF16 = mybir.dt.bfloat16
Act = mybir.ActivationFunctionType
Alu = mybir.AluOpType

NP_BF16 = np.dtype(ml_dtypes.bfloat16)

B, C, CK = 2, 256, 32
N_TOK = 4096
QCH = 1024
SCALE = (256 // 4) ** (-0.5)
N_CORES = 8

CT = C // 128           # 2 c-tiles of 128
KT = N_TOK // 128       # 32 key tiles of 128
QT = QCH // 512         # 2 query tiles of 512 per core
EG = 3                  # exp group: k-tiles per ScalarE exp instruction
N_WARM = int(os.environ.get("KERNEL_WARMUP", "8"))
N_WARM_MID = int(os.environ.get("KERNEL_WARMUP_MID", "3"))


def _install_ntff_hook():
    try:
        import antenv.axon_hooks  # noqa: F401
        return True
    except ImportError:
        pass
    try:
        import antenv
        mod = types.ModuleType("antenv.axon_hooks")
        _hook = [None]
        mod.set_axon_ntff_profile_hook = lambda h: _hook.__setitem__(0, h)
        mod.get_axon_ntff_profile_hook = lambda: _hook[0]
        sys.modules["antenv.axon_hooks"] = mod
        antenv.axon_hooks = mod
        from trn_agent_boot.trn_boot import _ntff_profile_via_ctypes
        mod.set_axon_ntff_profile_hook(
            _ntff_profile_via_ctypes("/opt/axon/libaxon_pjrt.so")
        )
        return True
    except Exception:
        return False


def _build_program():
    nc = bacc.Bacc(
        "TRN2", target_bir_lowering=False, debug=False, num_devices=N_CORES
    )
    xq = nc.dram_tensor("xq", [C, QCH], F32, kind="ExternalInput").ap()
    xkf = nc.dram_tensor("xkf", [128, N_TOK], F32, kind="ExternalInput").ap()
    xkt = nc.dram_tensor("xkt", [128, KT, 128], BF16, kind="ExternalInput").ap()
    wg = nc.dram_tensor("wg", [128, CT, 128], F32, kind="ExternalInput").ap()
    wup = nc.dram_tensor("wup", [128, CT, 128], BF16, kind="ExternalInput").ap()
    bg = nc.dram_tensor("bg", [CK], F32, kind="ExternalInput").ap()
    boe = nc.dram_tensor("boe", [C], F32, kind="ExternalInput").ap()
    ident = nc.dram_tensor("ident", [128, 128], F32, kind="ExternalInput").ap()
    out = nc.dram_tensor("out", [128, CT, QCH], F32, kind="ExternalOutput").ap()
    out_r = out
    xq_r = xq.rearrange("(t p) q -> p t q", p=128).bitcast(F32R)

    groups = []  # (k0, ks) per exp group
    k0 = 0
    while k0 < KT:
        ks = min(EG, KT - k0)
        groups.append((k0, ks))
        k0 += ks
    NG = len(groups)

    with tile.TileContext(nc) as tc:
        with tc.tile_pool(name="singles", bufs=1) as singles:
            xq_sb = singles.tile([128, CT, QCH], F32R)
            xk_sb = singles.tile([128, N_TOK], F32R)
            xkt_sb = singles.tile([128, KT, 128], BF16)
            wg_sb = singles.tile([128, CT, 128], F32R)
            wup_sb = singles.tile([128, CT, 128], BF16)
            bg_sb = singles.tile([CK, 1], F32)
            boe_sb = singles.tile([128, CT], F32)
            g_sb = singles.tile([128, QCH], F32R)
            zf = singles.tile([128, QCH], F32)
            wj = singles.tile([128, 256], BF16)
            sel_f = singles.tile([128, 128], F32)
            sel_b = singles.tile([128, 128], BF16)
            warm_in = singles.tile([1, 8], F32)
            warm_e = singles.tile([1, 8], F32)
            z_sb = singles.tile([128, QT, 512], BF16)
            zn_sb = singles.tile([128, 512], BF16)
            r33 = singles.tile([CK + 1, 512], F32)
            id_sb = singles.tile([128, 128], F32R)
            res_sb = singles.tile([128, CT, QCH], F32R)

            # Constants; zf zero-fills the padded rows of g_sb (memset can't
            # write f32r directly -- ISA restriction -- a DVE copy can).
            nc.vector.memset(wj, 0.0)
            nc.vector.memset(zf, 0.0)
            nc.vector.memset(sel_f, 0.0)
            nc.vector.memset(sel_f[CK:CK + 1, :], 1.0)
            nc.vector.memset(warm_in, 0.0)
            nc.vector.tensor_copy(g_sb, zf)
            nc.vector.tensor_copy(zn_sb, zf.bitcast(BF16)[:, 0:512])
            nc.vector.tensor_copy(sel_b, sel_f)

            # Input DMAs ordered by first use; they drain roughly in
            # issue order at ~110GB/s, so critical-path bytes are minimized
            # and the first xk/xkt chunks are small.
            nc.sync.dma_start(out=bg_sb, in_=bg[:, None])
            nc.sync.dma_start(out=wg_sb, in_=wg.bitcast(F32R))
            for ci in range(CT):
                nc.sync.dma_start(
                    out=xq_sb[:, ci, 0:512], in_=xq_r[:, ci, 0:512]
                )
            nc.sync.dma_start(
                out=xk_sb[:, 0:256], in_=xkf[:, 0:256].bitcast(F32R)
            )
            nc.sync.dma_start(out=xkt_sb[:, 0:4, :], in_=xkt[:, 0:4, :])
            nc.sync.dma_start(
                out=xk_sb[:, 256:1024], in_=xkf[:, 256:1024].bitcast(F32R)
            )
            nc.sync.dma_start(out=xkt_sb[:, 4:8, :], in_=xkt[:, 4:8, :])
            for ci in range(CT):
                nc.sync.dma_start(
                    out=xq_sb[:, ci, 512:1024], in_=xq_r[:, ci, 512:1024]
                )
            for kb in range(1, 4):
                ks_ = slice(kb * 1024, (kb + 1) * 1024)
                nc.sync.dma_start(
                    out=xk_sb[:, ks_], in_=xkf[:, ks_].bitcast(F32R)
                )
                kt_ = slice(kb * 8, (kb + 1) * 8)
                nc.sync.dma_start(out=xkt_sb[:, kt_, :], in_=xkt[:, kt_, :])
            nc.sync.dma_start(out=wup_sb, in_=wup)
            nc.sync.dma_start(out=boe_sb, in_=boe.rearrange("(t p) -> p t", p=128))
            nc.sync.dma_start(out=id_sb, in_=ident.bitcast(F32R))

            # ACT exp-table load (~2.7us) overlaps the DMA-in phase.
            nc.scalar.activation(out=warm_e, in_=warm_in, func=Act.Exp)

            with tc.tile_pool(name="sps", bufs=2, space="PSUM") as sps, \
                 tc.tile_pool(name="zps", bufs=1, space="PSUM") as zps, \
                 tc.tile_pool(name="eps", bufs=1, space="PSUM") as eps, \
                 tc.tile_pool(name="epool", bufs=3) as epool, \
                 tc.tile_pool(name="tpool", bufs=4) as tpool:

                # MM-bound bf16 warmup: the HAM clock-gate needs ~3.4us of
                # gapless PE activity before it opens to 2.4GHz, so the
                # burst uses the (idle) double-buffered sps banks -- a
                # single-bank ring serializes on the WAW drain and never
                # warms -- and seamlessly abuts G and the first S groups.
                def emit_warm(n):
                    for _ in range(n):
                        wm = sps.tile([128, EG, 512], F32, tag="s", name="wm")
                        nc.tensor.matmul(
                            wm[:, 0, 0:256], lhsT=wj[:, 0:128], rhs=wj,
                            start=True, stop=True, skip_group_check=True,
                        )

                def emit_g(qi):
                    # G = wg.T @ xq (+bg) -> g rows 0..31 (rows 32+ stay 0)
                    qsl = slice(qi * 512, (qi + 1) * 512)
                    pool, tg = (zps, "z") if qi == 0 else (eps, "e")
                    gp = pool.tile([128, 512], F32, tag=tg, name=f"gp{qi}")
                    for ci in range(CT):
                        nc.tensor.matmul(
                            gp, lhsT=wg_sb[:, ci, :],
                            rhs=xq_sb[:, ci, qsl],
                            start=(ci == 0), stop=(ci == CT - 1),
                        )
                    nc.vector.tensor_scalar(
                        out=g_sb[0:CK, qsl], in0=gp[0:CK, :],
                        scalar1=bg_sb, scalar2=None, op0=Alu.add,
                    )

                def emit_sgroup(qi, gi):
                    k0, ks = groups[gi]
                    qsl = slice(qi * 512, (qi + 1) * 512)
                    sp = sps.tile([128, EG, 512], F32, tag="s", name="sp")
                    for i in range(ks):
                        nc.tensor.matmul(
                            sp[:, i, :],
                            lhsT=xk_sb[:, (k0 + i) * 128:(k0 + i + 1) * 128],
                            rhs=g_sb[:, qsl],
                            start=True, stop=True, skip_group_check=True,
                        )
                    return sp

                def emit_expz(qi, gi, sp, z_ps):
                    k0, ks = groups[gi]
                    e = epool.tile([128, EG, 512], BF16, tag="e", name="e")
                    nc.scalar.activation(
                        out=e[:, 0:ks, :], in_=sp[:, 0:ks, :], func=Act.Exp
                    )
                    for i in range(ks):
                        nc.tensor.matmul(
                            z_ps,
                            lhsT=xkt_sb[:, k0 + i, :],
                            rhs=e[:, i, :],
                            start=(k0 + i == 0), stop=(k0 + i == KT - 1),
                            skip_group_check=True,
                        )

                def emit_epilogue(qi, z_ps):
                    qsl = slice(qi * 512, (qi + 1) * 512)
                    last = qi == QT - 1
                    # z rows 33..127 are exact zeros (zero-padded stationary)
                    nc.vector.tensor_copy(z_sb[:, qi, :], z_ps)
                    d128 = eps.tile([128, 512], F32, tag="e", name="d128")
                    nc.tensor.matmul(
                        d128, lhsT=sel_b, rhs=z_sb[:, qi, :],
                        start=True, stop=True, skip_group_check=True,
                    )
                    nc.vector.reciprocal_approx_fast(
                        out=r33, in_=d128[0:CK + 1, :]
                    )
                    # normalize the rank-33 z once instead of the two
                    # 128-wide y tiles (zn rows 33..127 stay zero)
                    nc.vector.tensor_mul(
                        zn_sb[0:CK + 1, :], z_sb[0:CK + 1, qi, :], r33,
                    )
                    for co in range(CT):
                        # Only the final tile's y1 may ride the zps bank: a
                        # mid-loop y1 there blocks the next q-tile's z
                        # accumulation (same bank) behind this epilogue.
                        pool, tg = (zps, "z") if (last and co == 1) else (eps, "e")
                        yp = pool.tile([128, 512], F32, tag=tg, name=f"y{co}")
                        nc.tensor.matmul(
                            yp, lhsT=wup_sb[:, co, :], rhs=zn_sb,
                            start=True, stop=False, skip_group_check=True,
                        )
                        # residual + bias accumulated into PSUM by the PE:
                        # y += I @ (xq + boe) -- the tail then needs only a
                        # single PSUM->SBUF copy, on ScalarE (idle once the
                        # last exp has retired), not the busy DVE.
                        nc.tensor.matmul(
                            yp, lhsT=id_sb, rhs=res_sb[:, co, qsl],
                            start=False, stop=True, skip_group_check=True,
                        )
                        t = tpool.tile([128, 512], F32, tag=f"t{co}",
                                       name=f"t{co}")
                        if last:
                            nc.scalar.copy(t, yp)
                            for h in range(2):
                                hs = slice(h * 256, (h + 1) * 256)
                                hq = slice(qi * 512 + h * 256,
                                           qi * 512 + (h + 1) * 256)
                                nc.sync.dma_start(
                                    out=out_r[:, co, hq], in_=t[:, hs]
                                )
                        else:
                            nc.vector.tensor_copy(t, yp)
                            nc.sync.dma_start(out=out_r[:, co, qsl], in_=t)

                emit_warm(N_WARM)
                emit_g(0)
                emit_warm(N_WARM_MID)
                emit_g(1)
                for qi in range(QT):
                    qsl = slice(qi * 512, (qi + 1) * 512)
                    for co in range(CT):
                        nc.vector.tensor_scalar(
                            out=res_sb[:, co, qsl],
                            in0=xq_sb[:, co, qsl].bitcast(F32),
                            scalar1=boe_sb[:, co:co + 1], scalar2=None,
                            op0=Alu.add,
                        )
                # flat (qi, gi) schedule with one-group lookahead across the
                # q-tile boundary so ACT never drains at the transition
                seq = [(qi, gi) for qi in range(QT) for gi in range(NG)]
                zt = {}
                zt[0] = zps.tile([128, 512], F32, tag="z", name="z0")
                sp = emit_sgroup(*seq[0])
                for idx, (qi, gi) in enumerate(seq):
                    if idx + 1 < len(seq):
                        nqi, ngi = seq[idx + 1]
                        if ngi == 0:
                            zt[nqi] = zps.tile(
                                [128, 512], F32, tag="z", name=f"z{nqi}"
                            )
                        sp_next = emit_sgroup(nqi, ngi)
                    emit_expz(qi, gi, sp, zt[qi])
                    if idx + 1 < len(seq):
                        sp = sp_next
                    # previous q-tile's epilogue right after this q-tile's
                    # first exp is queued
                    if gi == 0 and qi > 0:
                        emit_epilogue(qi - 1, zt[qi - 1])
                emit_epilogue(QT - 1, zt[QT - 1])

    nc.compile()
    return nc


_NC = None


def _get_nc():
    global _NC
    if _NC is None:
        _NC = _build_program()
    return _NC


def kernel(F_VNet, F_Knowledge, Wq, bq, Wk, bk, Wv, bv, Wo, bo):
    F_VNet = np.asarray(F_VNet, dtype=np.float32)
    F_Knowledge = np.asarray(F_Knowledge, dtype=np.float32)
    Wq, bq = np.asarray(Wq, np.float32), np.asarray(bq, np.float32)
    Wv, bv = np.asarray(Wv, np.float32), np.asarray(bv, np.float32)
    Wk = np.asarray(Wk, np.float32)
    Wo, bo = np.asarray(Wo, np.float32), np.asarray(bo, np.float32)

    in_shape = F_VNet.shape
    xq_full = F_VNet.reshape(B, C, N_TOK)
    xk_full = F_Knowledge.reshape(B, CK, N_TOK)

    wg_h = (SCALE * Wq.T.astype(np.float64) @ Wk.astype(np.float64)).astype(
        np.float32
    )  # [256, 32]
    wu_h = (Wv.T.astype(np.float64) @ Wo.T.astype(np.float64)).astype(
        np.float32
    )  # [32, 256]
    bg_h = np.ascontiguousarray(SCALE * (Wk.T @ bq))
    boe_h = np.ascontiguousarray((bo + Wo @ bv).reshape(CT, 128).T)  # [128, CT]

    ident_h = np.ascontiguousarray(np.eye(128, dtype=np.float32))
    wg_pad = np.zeros((128, CT, 128), np.float32)
    wg_pad[:, :, 0:CK] = wg_h.reshape(CT, 128, CK).transpose(1, 0, 2)
    wup_pad = np.zeros((128, CT, 128), NP_BF16)
    for co in range(CT):
        wup_pad[0:CK, co, :] = wu_h[:, co * 128:(co + 1) * 128]

    per_b = {}
    for b in range(B):
        xk_pad = np.zeros((128, N_TOK), np.float32)
        xk_pad[0:CK, :] = xk_full[b]
        xkt_pad = np.zeros((128, KT, 128), NP_BF16)
        # [p, ki, j] = xk[j, ki*128+p] for j<32; 1.0 at j==32
        xkt_pad[:, :, 0:CK] = (
            xk_full[b].T.reshape(KT, 128, CK).transpose(1, 0, 2)
        )
        xkt_pad[:, :, CK] = 1.0
        per_b[b] = (xk_pad, np.ascontiguousarray(xkt_pad))

    in_maps = []
    for core in range(N_CORES):
        b, j = divmod(core, N_CORES // B)
        xk_pad, xkt_pad = per_b[b]
        in_maps.append({
            "xq": np.ascontiguousarray(xq_full[b, :, j * QCH:(j + 1) * QCH]),
            "xkf": xk_pad, "xkt": xkt_pad,
            "wg": wg_pad, "wup": wup_pad,
            "bg": bg_h, "boe": boe_h, "ident": ident_h,
        })

    trace = bool(os.environ.get("KERNEL_TRACE"))
    if trace:
        _install_ntff_hook()
    nc = _get_nc()
    res = run_bass_kernel_spmd(
        nc, in_maps, core_ids=list(range(N_CORES)), trace=trace
    )
    kernel.last_results = res

    out = np.empty((B, C, N_TOK), np.float32)
    for core in range(N_CORES):
        b, j = divmod(core, N_CORES // B)
        # device layout [128, CT, QCH] -> [C, QCH]
        o = res.results[core]["out"].transpose(1, 0, 2).reshape(C, QCH)
        out[b, :, j * QCH:(j + 1) * QCH] = o
    return out.reshape(in_shape)


# revision 20
# speedup vs baseline: 1.7404x; 1.0284x over previous
"""Trainium2 Bass kernel for nn_CrossAttentionFusion.

Math (per batch b), feature-major on device:
    xq = F_VNet[b]      [C=256, N=4096]   (native layout)
    xk = F_Knowledge[b] [32, 4096]
    g  = wg.T @ xq + bg            [32, Nq]   wg = SCALE*(Wq.T@Wk) [256,32],
                                              bg = SCALE*(Wk.T@bq) [32]
    S[k,q] = sum_j xk[j,k] g[j,q]  (flash [k,q] layout; bk softmax-invariant)
    E = exp(S)  (no max-subtraction; |S| small)
    Z' = [xkT | 1]-contract:  Z'[j,q] = sum_k xkT[k,j] E[k,q]  for j<32,
         Z'[32,q] = d[q] = sum_k E[k,q]  (ones column -> denominator for free)
    out = (wu.T @ Z) * (1/d) + boe + xq,  wu = Wv.T@Wo.T, boe = bo + Wo@bv

Perf structure (vs the 112.7us 4-matmul/k-tile version):
  - 2 matmuls per k-tile (S f32r, Z bf16); the 256-wide output projection is
    applied once per q-tile to the rank-32 Z instead of every k-tile to E.
  - d rides as a 33rd stationary column of Z; broadcast to 128 partitions
    with one selector matmul, reciprocal via the fast DVE approx.
  - exp batched 3 PSUM banks per ScalarE instruction (FD=1536); ScalarE is
    the ~31us floor and gates the steady state.
  - contractions zero-padded K=32->128: every matmul runs in the (128,128)
    PE tile mode, no mode-switch drains.
  - bf16 N=256 warmup burst (MM-bound, so the HAM clock-gate actually goes
    to 8/8 -- an f32r burst is LDWEIGHTS-bound and stays cold).
  - DMAs ordered by first use; G for the second q-tile is emitted one group
    late so its xq never stalls the PE queue.

Sharding: 8 cores = batch(2) x query-chunk(4 x 1024 tokens); K/V replicated
within a batch group; host slices inputs / folds weights / gathers outputs.
"""

import os
import sys
import types

import numpy as np

for _p in (
    "/root/.axon_site",
    "/root/.axon_site/_ro/trn_rl_repo",
    "/root/.axon_site/_ro/pypackages",
    "/opt/trn_rl_repo",
):
    if os.path.isdir(_p) and _p not in sys.path:
        sys.path.append(_p)

import ml_dtypes  # noqa: E402

import concourse.bass as bass  # noqa: E402,F401
import concourse.tile as tile  # noqa: E402
from concourse import bacc, mybir  # noqa: E402
from concourse.bass_utils import run_bass_kernel_spmd  # noqa: E402

F32 = mybir.dt.float32
F32R = mybir.dt.float32r
BF16 = mybir.dt.bfloat16
Act = mybir.ActivationFunctionType
Alu = mybir.AluOpType

NP_BF16 = np.dtype(ml_dtypes.bfloat16)

B, C, CK = 2, 256, 32
N_TOK = 4096
QCH = 1024
SCALE = (256 // 4) ** (-0.5)
N_CORES = 8

CT = C // 128           # 2 c-tiles of 128
KT = N_TOK // 128       # 32 key tiles of 128
QT = QCH // 512         # 2 query tiles of 512 per core
EG = 3                  # exp group: k-tiles per ScalarE exp instruction
N_WARM = int(os.environ.get("KERNEL_WARMUP", "8"))
N_WARM_MID = int(os.environ.get("KERNEL_WARMUP_MID", "3"))


def _install_ntff_hook():
    try:
        import antenv.axon_hooks  # noqa: F401
        return True
    except ImportError:
        pass
    try:
        import antenv
        mod = types.ModuleType("antenv.axon_hooks")
        _hook = [None]
        mod.set_axon_ntff_profile_hook = lambda h: _hook.__setitem__(0, h)
        mod.get_axon_ntff_profile_hook = lambda: _hook[0]
        sys.modules["antenv.axon_hooks"] = mod
        antenv.axon_hooks = mod
        from trn_agent_boot.trn_boot import _ntff_profile_via_ctypes
        mod.set_axon_ntff_profile_hook(
            _ntff_profile_via_ctypes("/opt/axon/libaxon_pjrt.so")
        )
        return True
    except Exception:
        return False


def _build_program():
    nc = bacc.Bacc(
        "TRN2", target_bir_lowering=False, debug=False, num_devices=N_CORES
    )
    xq = nc.dram_tensor("xq", [C, QCH], F32, kind="ExternalInput").ap()
    xkf = nc.dram_tensor("xkf", [CK, N_TOK], F32, kind="ExternalInput").ap()
    xkt = nc.dram_tensor("xkt", [128, KT, 128], BF16, kind="ExternalInput").ap()
    wg = nc.dram_tensor("wg", [128, CT, 128], F32, kind="ExternalInput").ap()
    wup = nc.dram_tensor("wup", [128, CT, 128], BF16, kind="ExternalInput").ap()
    bg = nc.dram_tensor("bg", [CK], F32, kind="ExternalInput").ap()
    boe = nc.dram_tensor("boe", [C], F32, kind="ExternalInput").ap()
    ident = nc.dram_tensor("ident", [128, 128], F32, kind="ExternalInput").ap()
    out = nc.dram_tensor("out", [128, CT, QCH], F32, kind="ExternalOutput").ap()
    out_r = out
    xq_r = xq.rearrange("(t p) q -> p t q", p=128).bitcast(F32R)

    groups = []  # (k0, ks) per exp group
    k0 = 0
    while k0 < KT:
        ks = min(EG, KT - k0)
        groups.append((k0, ks))
        k0 += ks
    NG = len(groups)

    with tile.TileContext(nc) as tc:
        with tc.tile_pool(name="singles", bufs=1) as singles:
            xq_sb = singles.tile([128, CT, QCH], F32R)
            xk_sb = singles.tile([128, N_TOK], F32R)
            xkt_sb = singles.tile([128, KT, 128], BF16)
            wg_sb = singles.tile([128, CT, 128], F32R)
            wup_sb = singles.tile([128, CT, 128], BF16)
            bg_sb = singles.tile([CK, 1], F32)
            boe_sb = singles.tile([128, CT], F32)
            g_sb = singles.tile([128, QCH], F32R)
            zf = singles.tile([128, QCH], F32)
            wj = singles.tile([128, 256], BF16)
            sel_f = singles.tile([128, 128], F32)
            sel_b = singles.tile([128, 128], BF16)
            warm_in = singles.tile([1, 8], F32)
            warm_e = singles.tile([1, 8], F32)
            z_sb = singles.tile([128, QT, 512], BF16)
            zn_sb = singles.tile([128, 512], BF16)
            r33 = singles.tile([CK + 1, 512], F32)
            id_sb = singles.tile([128, 128], F32R)
            res_sb = singles.tile([128, CT, QCH], F32R)

            # Constants; zf zero-fills the padded rows of g_sb (memset can't
            # write f32r directly -- ISA restriction -- a DVE copy can).
            nc.vector.memset(wj, 0.0)
            nc.vector.memset(zf, 0.0)
            nc.vector.memset(sel_f, 0.0)
            nc.vector.memset(sel_f[CK:CK + 1, :], 1.0)
            nc.vector.memset(warm_in, 0.0)
            nc.vector.tensor_copy(g_sb, zf)
            nc.vector.tensor_copy(zn_sb, zf.bitcast(BF16)[:, 0:512])
            nc.vector.tensor_copy(sel_b, sel_f)

            # Input DMAs ordered by first use; they drain roughly in
            # issue order at ~110GB/s, so critical-path bytes are minimized
            # and the first xk/xkt chunks are small.
            nc.sync.dma_start(out=bg_sb, in_=bg[:, None])
            nc.sync.dma_start(out=wg_sb, in_=wg.bitcast(F32R))
            for ci in range(CT):
                nc.sync.dma_start(
                    out=xq_sb[:, ci, 0:512], in_=xq_r[:, ci, 0:512]
                )
            nc.sync.dma_start(
                out=xk_sb[0:CK, 0:1024], in_=xkf[:, 0:1024].bitcast(F32R)
            )
            nc.sync.dma_start(out=xkt_sb[:, 0:4, :], in_=xkt[:, 0:4, :])
            nc.sync.dma_start(out=xkt_sb[:, 4:8, :], in_=xkt[:, 4:8, :])
            for ci in range(CT):
                nc.sync.dma_start(
                    out=xq_sb[:, ci, 512:1024], in_=xq_r[:, ci, 512:1024]
                )
            for kb in range(1, 4):
                ks_ = slice(kb * 1024, (kb + 1) * 1024)
                nc.sync.dma_start(
                    out=xk_sb[0:CK, ks_], in_=xkf[:, ks_].bitcast(F32R)
                )
                kt_ = slice(kb * 8, (kb + 1) * 8)
                nc.sync.dma_start(out=xkt_sb[:, kt_, :], in_=xkt[:, kt_, :])
            nc.sync.dma_start(out=wup_sb, in_=wup)
            nc.sync.dma_start(out=boe_sb, in_=boe.rearrange("(t p) -> p t", p=128))
            nc.sync.dma_start(out=id_sb, in_=ident.bitcast(F32R))

            # ACT exp-table load (~2.7us) overlaps the DMA-in phase.
            nc.scalar.activation(out=warm_e, in_=warm_in, func=Act.Exp)

            with tc.tile_pool(name="sps", bufs=2, space="PSUM") as sps, \
                 tc.tile_pool(name="zps", bufs=1, space="PSUM") as zps, \
                 tc.tile_pool(name="eps", bufs=1, space="PSUM") as eps, \
                 tc.tile_pool(name="epool", bufs=3) as epool, \
                 tc.tile_pool(name="tpool", bufs=4) as tpool:

                # MM-bound bf16 warmup: the HAM clock-gate needs ~3.4us of
                # gapless PE activity before it opens to 2.4GHz, so the
                # burst uses the (idle) double-buffered sps banks -- a
                # single-bank ring serializes on the WAW drain and never
                # warms -- and seamlessly abuts G and the first S groups.
                def emit_warm(n):
                    for _ in range(n):
                        wm = sps.tile([128, EG, 512], F32, tag="s", name="wm")
                        nc.tensor.matmul(
                            wm[:, 0, 0:256], lhsT=wj[:, 0:128], rhs=wj,
                            start=True, stop=True, skip_group_check=True,
                        )

                def emit_g(qi):
                    # G = wg.T @ xq (+bg) -> g rows 0..31 (rows 32+ stay 0)
                    qsl = slice(qi * 512, (qi + 1) * 512)
                    pool, tg = (zps, "z") if qi == 0 else (eps, "e")
                    gp = pool.tile([128, 512], F32, tag=tg, name=f"gp{qi}")
                    for ci in range(CT):
                        nc.tensor.matmul(
                            gp, lhsT=wg_sb[:, ci, :],
                            rhs=xq_sb[:, ci, qsl],
                            start=(ci == 0), stop=(ci == CT - 1),
                        )
                    nc.vector.tensor_scalar(
                        out=g_sb[0:CK, qsl], in0=gp[0:CK, :],
                        scalar1=bg_sb, scalar2=None, op0=Alu.add,
                    )

                def emit_sgroup(qi, gi):
                    k0, ks = groups[gi]
                    qsl = slice(qi * 512, (qi + 1) * 512)
                    sp = sps.tile([128, EG, 512], F32, tag="s", name="sp")
                    for i in range(ks):
                        nc.tensor.matmul(
                            sp[:, i, :],
                            lhsT=xk_sb[:, (k0 + i) * 128:(k0 + i + 1) * 128],
                            rhs=g_sb[:, qsl],
                            start=True, stop=True, skip_group_check=True,
                        )
                    return sp

                def emit_expz(qi, gi, sp, z_ps):
                    k0, ks = groups[gi]
                    e = epool.tile([128, EG, 512], BF16, tag="e", name="e")
                    nc.scalar.activation(
                        out=e[:, 0:ks, :], in_=sp[:, 0:ks, :], func=Act.Exp
                    )
                    for i in range(ks):
                        nc.tensor.matmul(
                            z_ps,
                            lhsT=xkt_sb[:, k0 + i, :],
                            rhs=e[:, i, :],
                            start=(k0 + i == 0), stop=(k0 + i == KT - 1),
                            skip_group_check=True,
                        )

                def emit_epilogue(qi, z_ps):
                    qsl = slice(qi * 512, (qi + 1) * 512)
                    last = qi == QT - 1
                    # z rows 33..127 are exact zeros (zero-padded stationary)
                    nc.vector.tensor_copy(z_sb[:, qi, :], z_ps)
                    d128 = eps.tile([128, 512], F32, tag="e", name="d128")
                    nc.tensor.matmul(
                        d128, lhsT=sel_b, rhs=z_sb[:, qi, :],
                        start=True, stop=True, skip_group_check=True,
                    )
                    nc.vector.reciprocal_approx_fast(
                        out=r33, in_=d128[0:CK + 1, :]
                    )
                    # normalize the rank-33 z once instead of the two
                    # 128-wide y tiles (zn rows 33..127 stay zero)
                    nc.vector.tensor_mul(
                        zn_sb[0:CK + 1, :], z_sb[0:CK + 1, qi, :], r33,
                    )
                    for co in range(CT):
                        # Only the final tile's y1 may ride the zps bank: a
                        # mid-loop y1 there blocks the next q-tile's z
                        # accumulation (same bank) behind this epilogue.
                        pool, tg = (zps, "z") if (last and co == 1) else (eps, "e")
                        yp = pool.tile([128, 512], F32, tag=tg, name=f"y{co}")
                        nc.tensor.matmul(
                            yp, lhsT=wup_sb[:, co, :], rhs=zn_sb,
                            start=True, stop=False, skip_group_check=True,
                        )
                        # residual + bias accumulated into PSUM by the PE:
                        # y += I @ (xq + boe) -- the tail then needs only a
                        # single PSUM->SBUF copy, on ScalarE (idle once the
                        # last exp has retired), not the busy DVE.
                        nc.tensor.matmul(
                            yp, lhsT=id_sb, rhs=res_sb[:, co, qsl],
                            start=False, stop=True, skip_group_check=True,
                        )
                        t = tpool.tile([128, 512], F32, tag=f"t{co}",
                                       name=f"t{co}")
                        if last:
                            nc.scalar.copy(t, yp)
                            for h in range(2):
                                hs = slice(h * 256, (h + 1) * 256)
                                hq = slice(qi * 512 + h * 256,
                                           qi * 512 + (h + 1) * 256)
                                nc.sync.dma_start(
                                    out=out_r[:, co, hq], in_=t[:, hs]
                                )
                        else:
                            nc.vector.tensor_copy(t, yp)
                            nc.sync.dma_start(out=out_r[:, co, qsl], in_=t)

                emit_warm(N_WARM)
                emit_g(0)
                emit_warm(N_WARM_MID)
                emit_g(1)
                for qi in range(QT):
                    qsl = slice(qi * 512, (qi + 1) * 512)
                    for co in range(CT):
                        nc.vector.tensor_scalar(
                            out=res_sb[:, co, qsl],
                            in0=xq_sb[:, co, qsl].bitcast(F32),
                            scalar1=boe_sb[:, co:co + 1], scalar2=None,
                            op0=Alu.add,
                        )
                # flat (qi, gi) schedule with one-group lookahead across the
                # q-tile boundary so ACT never drains at the transition
                seq = [(qi, gi) for qi in range(QT) for gi in range(NG)]
                zt = {}
                zt[0] = zps.tile([128, 512], F32, tag="z", name="z0")
                sp = emit_sgroup(*seq[0])
                for idx, (qi, gi) in enumerate(seq):
                    if idx + 1 < len(seq):
                        nqi, ngi = seq[idx + 1]
                        if ngi == 0:
                            zt[nqi] = zps.tile(
                                [128, 512], F32, tag="z", name=f"z{nqi}"
                            )
                        sp_next = emit_sgroup(nqi, ngi)
                    emit_expz(qi, gi, sp, zt[qi])
                    if idx + 1 < len(seq):
                        sp = sp_next
                    # previous q-tile's epilogue right after this q-tile's
                    # first exp is queued
                    if gi == 0 and qi > 0:
                        emit_epilogue(qi - 1, zt[qi - 1])
                emit_epilogue(QT - 1, zt[QT - 1])

    nc.compile()
    return nc


_NC = None


def _get_nc():
    global _NC
    if _NC is None:
        _NC = _build_program()
    return _NC


def kernel(F_VNet, F_Knowledge, Wq, bq, Wk, bk, Wv, bv, Wo, bo):
    F_VNet = np.asarray(F_VNet, dtype=np.float32)
    F_Knowledge = np.asarray(F_Knowledge, dtype=np.float32)
    Wq, bq = np.asarray(Wq, np.float32), np.asarray(bq, np.float32)
    Wv, bv = np.asarray(Wv, np.float32), np.asarray(bv, np.float32)
    Wk = np.asarray(Wk, np.float32)
    Wo, bo = np.asarray(Wo, np.float32), np.asarray(bo, np.float32)

    in_shape = F_VNet.shape
    xq_full = F_VNet.reshape(B, C, N_TOK)
    xk_full = F_Knowledge.reshape(B, CK, N_TOK)

    wg_h = (SCALE * Wq.T.astype(np.float64) @ Wk.astype(np.float64)).astype(
        np.float32
    )  # [256, 32]
    wu_h = (Wv.T.astype(np.float64) @ Wo.T.astype(np.float64)).astype(
        np.float32
    )  # [32, 256]
    bg_h = np.ascontiguousarray(SCALE * (Wk.T @ bq))
    boe_h = np.ascontiguousarray((bo + Wo @ bv).reshape(CT, 128).T)  # [128, CT]

    ident_h = np.ascontiguousarray(np.eye(128, dtype=np.float32))
    wg_pad = np.zeros((128, CT, 128), np.float32)
    wg_pad[:, :, 0:CK] = wg_h.reshape(CT, 128, CK).transpose(1, 0, 2)
    wup_pad = np.zeros((128, CT, 128), NP_BF16)
    for co in range(CT):
        wup_pad[0:CK, co, :] = wu_h[:, co * 128:(co + 1) * 128]

    per_b = {}
    for b in range(B):
        xk_pad = np.ascontiguousarray(xk_full[b])
        xkt_pad = np.zeros((128, KT, 128), NP_BF16)
        # [p, ki, j] = xk[j, ki*128+p] for j<32; 1.0 at j==32
        xkt_pad[:, :, 0:CK] = (
            xk_full[b].T.reshape(KT, 128, CK).transpose(1, 0, 2)
        )
        xkt_pad[:, :, CK] = 1.0
        per_b[b] = (xk_pad, np.ascontiguousarray(xkt_pad))

    in_maps = []
    for core in range(N_CORES):
        b, j = divmod(core, N_CORES // B)
        xk_pad, xkt_pad = per_b[b]
        in_maps.append({
            "xq": np.ascontiguousarray(xq_full[b, :, j * QCH:(j + 1) * QCH]),
            "xkf": xk_pad, "xkt": xkt_pad,
            "wg": wg_pad, "wup": wup_pad,
            "bg": bg_h, "boe": boe_h, "ident": ident_h,
        })

    trace = bool(os.environ.get("KERNEL_TRACE"))
    if trace:
        _install_ntff_hook()
    nc = _get_nc()
    res = run_bass_kernel_spmd(
        nc, in_maps, core_ids=list(range(N_CORES)), trace=trace
    )
    kernel.last_results = res

    out = np.empty((B, C, N_TOK), np.float32)
    for core in range(N_CORES):
        b, j = divmod(core, N_CORES // B)
        # device layout [128, CT, QCH] -> [C, QCH]
        o = res.results[core]["out"].transpose(1, 0, 2).reshape(C, QCH)
        out[b, :, j * QCH:(j + 1) * QCH] = o
    return out.reshape(in_shape)
